# revision 3
# baseline (speedup 1.0000x reference)
"""Trainium2 Bass kernel for nn_E2EGuidedFilter (guided filter, r=8, eps=0.01).

Full inputs x, y: (8, 3, 1024, 1024) fp32. Data-parallel: one image per
NeuronCore (8 cores).

All-matmul structure (per channel, H=W=1024, 8 partition-blocks of 128):
  - host-prepped f16 inputs (layout A): xc = x-0.5, yA = y,
    xyp = (x-0.5)*y, xx = (x-0.5)^2. One banded window matrix wq (1/count
    folded) serves all four directional passes (H==W).
  - stage 1: H-matmul (A->B, qh folded) -> mid evac f16 -> W-matmul
    (B->A, qw folded) -> PSUM z_x, z_y, z_xy, z_xx per hc-block.
  - pointwise (layout A, fully normalized z): mx/my evacs on ACT;
    t1/s2/j1/b as fused scalar_tensor_tensor on GpSimd(Pool);
    num/den/recip/a on DVE (den fuses +eps via stt).
  - stage 2: H-matmul of a, b (A->B) -> amid (ACT) / bmid (DVE) ->
    W-matmul (B->A) -> combine out = z2a*xc + z2b with xc chunks
    re-DMA'd; output leaves in natural layout A (no transposes
    anywhere, no xcbq input).
  - stage 2 of channel ch is emitted after channel ch+1's loads for
    cross-channel overlap; big tiles ride a single ring pool.
"""

import os
import sys

import numpy as np

for _p in ("/opt/trn_rl_repo", "/root/.axon_site/_ro/trn_rl_repo"):
    if os.path.isdir(_p) and _p not in sys.path:
        sys.path.append(_p)

R = 8
EPS = 0.01
H = W = 1024
PB = H // 128  # 8 partition blocks
C = 3
NCORES = 8

_CACHE = {}


def _counts():
    i = np.arange(H)
    return (np.minimum(i + R, H - 1) - np.maximum(i - R, 0) + 1).astype(np.float64)


def _host_consts():
    qh = (1.0 / _counts()).astype(np.float32)

    def band_block(c, lo, n):
        Wt = np.zeros((128, n), np.float32)
        for j in range(n):
            hp = lo + j
            k0 = max(0, hp - R - 128 * c)
            k1 = min(127, hp + R - 128 * c)
            if k0 <= k1:
                Wt[k0 : k1 + 1, j] = qh[hp]
        return Wt

    W0 = band_block(0, 0, 136)
    Wi = band_block(1, 120, 144)
    W7 = band_block(7, 888, 136)
    wq = np.concatenate([W0, Wi, W7], axis=1).astype(np.float16)  # [128,416]
    return wq


def _mm_windows():
    halves = [[], []]
    for c in range(PB):
        lo = max(0, 128 * c - 8)
        hi = min(1024, 128 * c + 136)
        if c == 0:
            wt, wbase = "e0", 0
        elif c == PB - 1:
            wt, wbase = "e7", 888
        else:
            wt, wbase = "int", 128 * c - 8
        for hf in (0, 1):
            blo, bhi = 512 * hf, 512 * hf + 512
            s, e = max(lo, blo), min(hi, bhi)
            if s < e:
                halves[hf].append((c, s, e, wt, s - wbase, e - wbase))
    return halves


_HALVES = _mm_windows()


def _split_multi_waits(nc, mybir):
    """This container's walrus supports 1 sync wait per instruction (2 for
    EventSemaphore); Tile emits more. Move excess waits onto NoOps inserted
    just before the instruction on the same engine."""
    uid = [0]
    for f in nc.m.functions:
        for bb in f.blocks:
            out = []
            changed = False
            for inst in bb.instructions:
                si = inst.sync_info
                waits = list(si.on_wait) if si and si.on_wait else []
                cap = 2 if type(inst).__name__ == "InstEventSemaphore" else 1
                if len(waits) > cap:
                    for w in waits[:-cap]:
                        uid[0] += 1
                        nop = mybir.InstNoOp(name=f"wsplit-{uid[0]}", ins=[], outs=[])
                        nop.engine = inst.engine
                        nop.sync_info = mybir.SyncInfo(on_wait=[w], on_update=[])
                        out.append(nop)
                    si.on_wait = waits[-cap:]
                    changed = True
                out.append(inst)
            if changed:
                bb.instructions = out
    return nc


def _build_bass():
    import concourse.bass as bass
    import concourse.mybir as mybir
    from concourse import tile
    from contextlib import ExitStack

    f16 = mybir.dt.float16
    f32 = mybir.dt.float32
    AF = mybir.ActivationFunctionType
    OP = mybir.AluOpType

    nc = bass.Bass("TRN2", target_bir_lowering=False, debug=False)

    xc_d = nc.dram_tensor("xc", [C, PB, 128, W], f16, kind="ExternalInput").ap()
    y_d = nc.dram_tensor("yA", [C, PB, 128, W], f16, kind="ExternalInput").ap()
    xy_d = nc.dram_tensor("xyp", [C, PB, 128, W], f16, kind="ExternalInput").ap()
    xx_d = nc.dram_tensor("xx", [C, PB, 128, W], f16, kind="ExternalInput").ap()
    wq_d = nc.dram_tensor("wq", [128, 416], f16, kind="ExternalInput").ap()
    out_d = nc.dram_tensor("out", [C, PB, 128, W], f16, kind="ExternalOutput").ap()

    with tile.TileContext(nc) as tc, ExitStack() as ctx:
        pconst = ctx.enter_context(tc.tile_pool(name="const", bufs=1))
        wq_t = pconst.tile([128, 416], f16, tag="wq")
        nc.sync.dma_start(wq_t[:], wq_d[:])

        def wslice(wt, a, b):
            if wt == "e0":
                return wq_t[:, a:b]
            if wt == "int":
                return wq_t[:, 136 + a : 136 + b]
            return wq_t[:, 280 + a : 280 + b]

        # ---- pools ----
        pbig = ctx.enter_context(tc.tile_pool(name="big", bufs=10))
        pxq = ctx.enter_context(tc.tile_pool(name="xq", bufs=3))
        psm = ctx.enter_context(tc.tile_pool(name="sm", bufs=2))  # mx
        psm2 = ctx.enter_context(tc.tile_pool(name="sm2", bufs=2))  # my
        psm3 = ctx.enter_context(tc.tile_pool(name="sm3", bufs=2))  # t1/num
        psm4 = ctx.enter_context(tc.tile_pool(name="sm4", bufs=2))  # s2/den/r
        psm5 = ctx.enter_context(tc.tile_pool(name="sm5", bufs=2))  # j1
        psm6 = ctx.enter_context(tc.tile_pool(name="sm6", bufs=2))  # a
        psm7 = ctx.enter_context(tc.tile_pool(name="sm7", bufs=2))  # t / s2b
        pout = ctx.enter_context(tc.tile_pool(name="outst", bufs=2))
        pz_h = ctx.enter_context(tc.tile_pool(name="zh", bufs=2, space="PSUM"))
        pz_w = ctx.enter_context(tc.tile_pool(name="zw", bufs=2, space="PSUM"))

        def mm_group_full(z, lhs_of):
            mms = []
            for hf in (0, 1):
                first_in_bank = True
                for c, s, e, wt, wa, wb in _HALVES[hf]:
                    mms.append(
                        (z[:, s:e], lhs_of(c), wslice(wt, wa, wb), first_in_bank)
                    )
                    first_in_bank = False
            for i, (o, l, r, st) in enumerate(mms):
                nc.tensor.matmul(
                    o, l, r,
                    start=st,
                    stop=(i == len(mms) - 1),
                    skip_group_check=True,
                )
            return z

        _bt = [0]

        def big_tile():
            _bt[0] += 1
            return pbig.tile([128, PB * W], f16, tag="big", name=f"big{_bt[0]}")

        def load_full(dst_ap, src_t, ch):
            for b in range(PB):
                nc.sync.dma_start(dst_ap[b], src_t[ch, b])

        def hmm_pass(src_big, evac):
            """H-matmul pass (layout A -> B) with per-m PSUM evac -> mid."""
            midt = big_tile()
            for m in range(PB):
                zh = pz_h.tile([128, W], f32, tag="zh")
                mm_group_full(
                    zh,
                    lambda c, _b=src_big, _m=m: _b[:, c * W + 128 * _m : c * W + 128 * _m + 128],
                )
                evac(midt[:, m * W : (m + 1) * W], zh)
            return midt

        def emit_stage2(pch, aB, bB):
            # stage 2a: H-matmul of a (evac ACT) and b (evac DVE)
            amid = hmm_pass(aB, lambda d, z: nc.scalar.activation(d, z[:], AF.Copy))
            bmid = hmm_pass(bB, lambda d, z: nc.vector.tensor_copy(d, z[:]))
            # stage 2b: W-matmul back to layout A + combine with xc chunks
            for hc in range(PB):
                z2a = pz_w.tile([128, W], f32, tag="zw")
                mm_group_full(
                    z2a,
                    lambda m, _s=amid, _hc=hc: _s[:, m * W + 128 * _hc : m * W + 128 * _hc + 128],
                )
                xq = pxq.tile([128, W], f16, tag="xq")
                nc.sync.dma_start(xq[:], xc_d[pch, hc])
                t = psm7.tile([128, W], f16, tag="t7")
                nc.vector.tensor_mul(t[:], z2a[:], xq[:])
                z2b = pz_w.tile([128, W], f32, tag="zw")
                mm_group_full(
                    z2b,
                    lambda m, _s=bmid, _hc=hc: _s[:, m * W + 128 * _hc : m * W + 128 * _hc + 128],
                )
                s2b = psm7.tile([128, W], f16, tag="t7")
                nc.scalar.activation(s2b[:], z2b[:], AF.Copy)
                ot = pout.tile([128, W], f16, tag="outst")
                nc.gpsimd.tensor_add(ot[:], t[:], s2b[:])
                nc.sync.dma_start(out_d[pch, hc], ot[:])

        prev_ph2 = None
        for ch in range(C):
            # ---- stage 0: loads (per-block DMAs) ----
            xc_big = big_tile()
            yA_big = big_tile()
            xy_big = big_tile()
            xx_big = big_tile()
            load_full(xc_big[:].rearrange("p (b q) -> b p q", q=W), xc_d, ch)
            load_full(yA_big[:].rearrange("p (b q) -> b p q", q=W), y_d, ch)
            load_full(xy_big[:].rearrange("p (b q) -> b p q", q=W), xy_d, ch)
            load_full(xx_big[:].rearrange("p (b q) -> b p q", q=W), xx_d, ch)
            if prev_ph2 is not None:
                emit_stage2(*prev_ph2)
                prev_ph2 = None

            # ---- stage 1a: H-matmul (A->B, qh folded) + evacs ----
            mids = {}
            mids["x"] = hmm_pass(
                xc_big, lambda d, z: nc.scalar.activation(d, z[:], AF.Copy)
            )
            mids["y"] = hmm_pass(
                yA_big, lambda d, z: nc.scalar.activation(d, z[:], AF.Copy)
            )
            mids["xy"] = hmm_pass(
                xy_big, lambda d, z: nc.scalar.activation(d, z[:], AF.Copy)
            )
            mids["xx"] = hmm_pass(xx_big, lambda d, z: nc.vector.tensor_copy(d, z[:]))

            # ---- stage 1b: W-matmul (B->A, qw folded) + pointwise ----
            aB = big_tile()
            bB = big_tile()
            for hc in range(PB):
                def wmm_m(t, _hc=hc):
                    z = pz_w.tile([128, W], f32, tag="zw")
                    mm_group_full(
                        z,
                        lambda m, _t=t: mids[_t][:, m * W + 128 * _hc : m * W + 128 * _hc + 128],
                    )
                    return z

                z_x = wmm_m("x")
                mx = psm.tile([128, W], f16, tag="mx")
                nc.scalar.activation(mx[:], z_x[:], AF.Copy)
                z_y = wmm_m("y")
                my = psm2.tile([128, W], f16, tag="my")
                nc.scalar.activation(my[:], z_y[:], AF.Copy)
                t1 = psm3.tile([128, W], f16, tag="t1")
                nc.vector.tensor_mul(t1[:], mx[:], my[:])
                s2 = psm4.tile([128, W], f16, tag="s2")
                nc.gpsimd.tensor_mul(s2[:], mx[:], mx[:])
                z_xy = wmm_m("xy")
                nc.vector.tensor_sub(t1[:], z_xy[:], t1[:])  # t1 <- num
                z_xx = wmm_m("xx")
                nc.vector.scalar_tensor_tensor(
                    s2[:], z_xx[:], EPS, s2[:], OP.add, OP.subtract
                )  # s2 <- den
                with nc.allow_low_precision(
                    reason="18-bit reciprocal ample for eps-regularized den"
                ):
                    nc.vector.reciprocal(s2[:], s2[:])  # s2 <- 1/den
                ac = aB[:, hc * W : (hc + 1) * W]
                nc.vector.tensor_mul(ac, t1[:], s2[:])  # a
                j1 = psm5.tile([128, W], f16, tag="j1")
                nc.vector.tensor_mul(j1[:], ac, mx[:])
                bc = bB[:, hc * W : (hc + 1) * W]
                nc.vector.tensor_sub(bc, my[:], j1[:])

            prev_ph2 = (ch, aB, bB)
        emit_stage2(*prev_ph2)

    _split_multi_waits(nc, mybir)
    return nc


def _get_bass():
    if "nc" not in _CACHE:
        _CACHE["nc"] = _build_bass()
    return _CACHE["nc"]


def kernel(x, y):
    x = np.asarray(x)
    y = np.asarray(y)
    from concourse.bass_utils import run_bass_kernel_spmd

    nc = _get_bass()
    wq = _host_consts()
    B = x.shape[0]
    xcf = (x - 0.5).astype(np.float16)
    yf = y.astype(np.float16)
    xypf = (xcf.astype(np.float32) * y).astype(np.float16)
    xxf = (xcf.astype(np.float32) ** 2).astype(np.float16)
    sh = (B, C, PB, 128, W)
    in_maps = [
        {
            "xc": xcf.reshape(sh)[i],
            "yA": yf.reshape(sh)[i],
            "xyp": xypf.reshape(sh)[i],
            "xx": xxf.reshape(sh)[i],
            "wq": wq,
        }
        for i in range(B)
    ]
    res = run_bass_kernel_spmd(nc, in_maps, core_ids=list(range(B)))
    out = np.stack(
        [res.results[i]["out"].reshape(C, H, W) for i in range(B)]
    )
    return np.ascontiguousarray(out).astype(np.float32)
